# revision 34
# baseline (speedup 1.0000x reference)
"""MOELinearDGLFractional Trainium2 kernel.

Data-parallel over systems: 8 cores x 64 systems (512 rows each).
All heavy tensors are bf16 (halves HBM traffic — memory-bound regime —
and runs PE at 1 cyc/row; fp32 accumulate in PSUM).

x is laid out transposed on the host (same byte count, free layout
choice): xt[i', s*1024 + (2j+h)*128 + q] = x[512 s + 4 q + j, 128 h + i'],
so each system's lhsT chunks stream straight from DRAM with contiguous
2KB per-partition lines and no on-device transpose pipeline.

Per system s (rows r = 512*s + 4*q + j, q=partition, j=0..3):
  - DMA xt tile [128, 1024]
  - fused matmul psum[q, (j%2)*256+o] += xt_k.T @ V[:, h*64+s, :]
    (k=2j+h; V = v3 [128(i'), hb, 256(o: 0-127 moe, 128-255 reg)])
  - DVE psum + bias -> out_sb (bf16), DMA out.
wmix (per-system mixed expert weights) produced on-PE in a prologue:
  one K=128-packed matmul per 8 (o,h)-chunks against block-diagonal coeff.
Regular Linear weights replicated into V's o=128..255 region by ScalarE.
"""

import sys

sys.path.insert(0, "/opt/trn_rl_repo")

import numpy as np

N_TOTAL = 262144
B = 512
E = 16
I_DIM = 256
O_MOE = 128
O_REG = 128
NCORES = 8
L = 512  # rows per system


def build_program(n_sys, reps=1):
    """reps>1 executes the full kernel (const DMAs + prologue + main loop)
    reps times inside one NEFF — used only for timing, where the slope over
    reps isolates true HW execution time from the 33-110ms axon dispatch
    latency (the neuronx_cc hook allows just one bass_exec custom call per
    XLA module, so repetition must live inside the BIR program)."""
    import concourse.bass as bass
    import concourse.mybir as mybir

    f32 = mybir.dt.float32
    bf16 = mybir.dt.bfloat16
    rows = n_sys * L
    hb = 2 * n_sys  # (h, b) combined dim of V
    nldw = 32  # number of K=128-packed ldw groups (256 chunks / 8)
    pw_n = 8 * n_sys  # prod psum free size per group (8 chunks x n_sys)

    BIO = 4  # systems per IO DMA (amortizes per-DMA fixed latency)
    nblk = n_sys // BIO
    NXB = 3  # xt_sb depth (blocks of BIO systems)
    NOB = 3  # o_sb depth (blocks)
    NPB = 4  # outp depth (2 banks each = all 8 PSUM banks)

    nc = bass.Bass()
    xt_d = nc.declare_dram_parameter("xt", [128, rows * 2], bf16, isOutput=False)
    wsb_d = nc.declare_dram_parameter("wsb", [128, 4096], bf16, isOutput=False)
    c8t_d = nc.declare_dram_parameter("c8t", [128, pw_n], bf16, isOutput=False)
    linwt_d = nc.declare_dram_parameter("linwt", [128, 256], bf16, isOutput=False)
    bias_d = nc.declare_dram_parameter("bias2", [128, 512], f32, isOutput=False)
    out = nc.declare_dram_parameter("out", [rows, 256], bf16, isOutput=True)

    # 4-system output blocks: per partition q, 4 contiguous-2KB segments
    # (one per system in the block) at 512-row stride
    ov4 = out.rearrange("(t sg q j) m -> t q sg (j m)", sg=BIO, q=128, j=4)

    from contextlib import ExitStack

    with ExitStack() as ctx:
        en = ctx.enter_context
        wsb = en(nc.sbuf_tensor("wsb_sb", [128, 4096], bf16))
        c8t = en(nc.sbuf_tensor("c8t_sb", [128, pw_n], bf16))
        linwt = en(nc.sbuf_tensor("linwt_sb", [128, 256], bf16))
        bias2 = en(nc.sbuf_tensor("bias2_sb", [128, 512], f32))
        # V: [i', hb, o(256: moe|reg)] — o contiguous so matmul moving
        # operands stream unit-stride
        v3 = en(nc.sbuf_tensor("v3_sb", [128, hb, 256], bf16))
        xt_sb = [
            en(nc.sbuf_tensor(f"xt_sb{i}", [128, BIO * 1024], bf16))
            for i in range(NXB)
        ]
        o_sb = [
            en(nc.sbuf_tensor(f"o_sb{i}", [128, BIO * 1024], bf16))
            for i in range(NOB)
        ]
        outp = [
            [en(nc.psum_tensor(f"outp{i}{k}", [128, 512], f32)) for k in range(2)]
            for i in range(NPB)
        ]

        sem_names = (
            ["cst", "mm", "dve", "pw", "pwe", "pve", "lws", "lwv"]
            + [f"xin{i}" for i in range(NXB)]
            + [f"dout{i}" for i in range(NOB)]
        )
        sems = {n: en(nc.semaphore(n)) for n in sem_names}
        cst_s, mm_s, dve_s, pw_s, pwe_s, pve_s = (
            sems[n] for n in ["cst", "mm", "dve", "pw", "pwe", "pve"]
        )
        lw_sems = {0: sems["lws"], 1: sems["lwv"]}
        lw_cnt = {0: 0, 1: 0}
        xin = [sems[f"xin{i}"] for i in range(NXB)]
        dout = [sems[f"dout{i}"] for i in range(NOB)]

        block = en(nc.Block())

        @block.sync
        def _(sync):
            for r in range(reps):
                if r > 0:
                    # rep r-1 fully drained (dve is the last const reader)
                    sync.wait_ge(dve_s, r * n_sys)
                sync.dma_start(out=wsb[:], in_=wsb_d[:]).then_inc(cst_s, 16)
                sync.dma_start(out=c8t[:], in_=c8t_d[:]).then_inc(cst_s, 16)
                sync.dma_start(out=linwt[:], in_=linwt_d[:]).then_inc(cst_s, 16)
                sync.dma_start(out=bias2[:], in_=bias_d[:]).then_inc(cst_s, 16)
                for tb in range(nblk):
                    t = r * nblk + tb
                    if t >= NXB:
                        # buffer free once the previous occupant's matmuls ran
                        sync.wait_ge(mm_s, BIO * (t - NXB + 1))
                    sync.dma_start(
                        out=xt_sb[t % NXB][:],
                        in_=xt_d[:, tb * BIO * 1024 : (tb + 1) * BIO * 1024],
                    ).then_inc(xin[t % NXB], 16)

        @block.tensor
        def _(tensor):
            for r in range(reps):
                # ---- prologue: produce V moe columns (mixed expert wts) ----
                tensor.wait_ge(cst_s, 64 * (r + 1))
                for g in range(nldw):
                    gg = r * nldw + g
                    if gg >= NPB:
                        # bank free once group gg-NPB was evacuated (even
                        # groups on ScalarE -> pwe, odd on VectorE -> pve)
                        k = gg - NPB
                        tensor.wait_ge(pwe_s if k % 2 == 0 else pve_s, k // 2 + 1)
                    if r > 0 and g < NPB:
                        # outp banks still read by rep r-1's bias adds
                        tensor.wait_ge(dve_s, r * n_sys)
                    pp = outp[gg % NPB][0]
                    for v in range(8):
                        inst = nc.tensor.matmul(
                            pp[:, v * n_sys : (v + 1) * n_sys],
                            wsb[:, g * 128 : (g + 1) * 128],
                            c8t[:, v * n_sys : (v + 1) * n_sys],
                            start=True,
                            stop=True,
                        )
                    inst.then_inc(pw_s, 1)

                # ---- main loop: 8 fused matmuls per system ----
                for s in range(n_sys):
                    g = r * n_sys + s
                    t = g // BIO
                    if g % BIO == 0:
                        tensor.wait_ge(xin[t % NXB], 16 * (t // NXB + 1))
                    if s == 0:
                        tensor.wait_ge(pwe_s, (r + 1) * nldw // 2)
                        tensor.wait_ge(pve_s, (r + 1) * nldw // 2)
                    if g >= NPB:
                        tensor.wait_ge(dve_s, g - (NPB - 1))
                    xb = xt_sb[t % NXB]
                    xo = (g % BIO) * 1024
                    for j in range(4):
                        pp = outp[g % NPB][j // 2]
                        for h in range(2):
                            inst = nc.tensor.matmul(
                                pp[:, (j % 2) * 256 : (j % 2) * 256 + 256],
                                xb[:, xo + (2 * j + h) * 128 : xo + (2 * j + h + 1) * 128],
                                v3[:, bass.ds(h * n_sys + s, 1), :],
                                start=(h == 0),
                                stop=(h == 1),
                            )
                    inst.then_inc(mm_s, 1)

        def linwt_fill(eng, cp, h):
            # seed + log-doubling of linwt into v3[:, h*n+b, 128:256] over b;
            # a same-engine semaphore chain orders each step's RAW hazard
            sem = lw_sems[h]
            cp(
                out=v3[:, bass.ds(h * n_sys, 1), 128:256],
                in_=linwt[:, h * 128 : (h + 1) * 128],
            ).then_inc(sem, 1)
            lw_cnt[h] += 1
            k = 1
            while k < n_sys:
                kk = min(k, n_sys - k)
                eng.wait_ge(sem, lw_cnt[h])
                cp(
                    out=v3[:, h * n_sys + k : h * n_sys + k + kk, 128:256],
                    in_=v3[:, h * n_sys : h * n_sys + kk, 128:256],
                ).then_inc(sem, 1)
                lw_cnt[h] += 1
                k *= 2

        def evac_group(cp, r, g):
            # group g holds chunks c=8g+v = 2o+h (o = 4g+o2): psum free
            # (o2, h, b) with (h, b) contiguous -> one dim-permuted copy
            pp = outp[(r * nldw + g) % NPB][0]
            return cp(
                out=v3[:, :, 4 * g : 4 * g + 4],
                in_=pp[:, 0 : 8 * n_sys].rearrange("p (o2 hb) -> p hb o2", o2=4),
            )

        @block.scalar
        def _(scalar):
            # prologue work is split with VectorE (idle until matmuls flow):
            # ScalarE takes the h=0 linwt fill + even evac groups
            for r in range(reps):
                scalar.wait_ge(cst_s, 64 * (r + 1))
                if r > 0:
                    # v3 still read by rep r-1's matmuls
                    scalar.wait_ge(mm_s, r * n_sys)
                linwt_fill(scalar, nc.scalar.copy, 0)
                for g in range(0, nldw, 2):
                    scalar.wait_ge(pw_s, r * nldw + g + 1)
                    evac_group(nc.scalar.copy, r, g).then_inc(pwe_s, 1)

        @block.vector
        def _(vector):
            for r in range(reps):
                # prologue: h=1 linwt fill + odd evac groups (v3 free: our
                # own rep r-1 adds already waited out all its matmuls)
                vector.wait_ge(cst_s, 64 * (r + 1))
                linwt_fill(vector, nc.vector.tensor_copy, 1)
                for g in range(1, nldw, 2):
                    vector.wait_ge(pw_s, r * nldw + g + 1)
                    evac_group(nc.vector.tensor_copy, r, g).then_inc(pve_s, 1)
                # main: bias adds
                for s in range(n_sys):
                    g = r * n_sys + s
                    vector.wait_ge(mm_s, g + 1)
                    t = g // BIO
                    if g % BIO == 0 and t >= NOB:
                        vector.wait_ge(dout[t % NOB], 16 * (t // NOB))
                    pb = g % NPB
                    ob = o_sb[t % NOB]
                    oo = (g % BIO) * 1024
                    nc.vector.tensor_add(
                        ob[:, oo : oo + 512], outp[pb][0][:], bias2[:]
                    )
                    nc.vector.tensor_add(
                        ob[:, oo + 512 : oo + 1024], outp[pb][1][:], bias2[:]
                    ).then_inc(dve_s, 1)

        @block.gpsimd
        def _(gpsimd):
            # main: out DMA per block of BIO systems
            tblk = reps * nblk
            for t in range(tblk):
                gpsimd.wait_ge(dve_s, BIO * (t + 1))
                gpsimd.dma_start(
                    out=ov4[t % nblk], in_=o_sb[t % NOB][:]
                ).then_inc(dout[t % NOB], 16)
            for i in range(NOB):
                gpsimd.wait_ge(dout[i], 16 * ((tblk - i + NOB - 1) // NOB))

    return nc


def _bf16():
    import ml_dtypes

    return ml_dtypes.bfloat16


def _host_inputs(x, coeff, moe_weights, moe_bias, lin_weight, lin_bias, n_sys, core):
    """Build per-core in_map."""
    bf = _bf16()
    # wsb: [16v+e, 128g+i'] = W[e, o(c), 128h(c)+i'], c=8g+v, (o,h)=divmod(c,2)
    Wr = np.asarray(moe_weights).reshape(E, 128, 2, 128)  # e,o,h,i'
    ch = Wr.transpose(1, 2, 0, 3).reshape(256, E, 128)  # c,e,i'
    wsb = np.ascontiguousarray(
        ch.reshape(32, 8, E, 128).transpose(1, 2, 0, 3).reshape(128, 4096)
    ).astype(bf)
    b0 = core * n_sys
    cT = np.asarray(coeff)[b0 : b0 + n_sys].T.astype(np.float32)  # [E, n_sys]
    c8t = np.zeros((128, 8 * n_sys), np.float32)
    for v in range(8):
        c8t[16 * v : 16 * v + E, v * n_sys : (v + 1) * n_sys] = cT
    c8t = c8t.astype(bf)
    lw = np.asarray(lin_weight)  # [128, 256]
    linwt = np.ascontiguousarray(
        lw.reshape(128, 2, 128).transpose(2, 1, 0).reshape(128, 256)
    ).astype(bf)
    bias_cat = np.concatenate([np.asarray(moe_bias), np.asarray(lin_bias)]).astype(
        np.float32
    )
    bias2 = np.tile(bias_cat, (128, 2))
    # xt[i', (s, j, h, q)] = x[512 s + 4 q + j, 128 h + i']
    xs = np.asarray(x)[core * n_sys * L : (core + 1) * n_sys * L]
    xt = np.ascontiguousarray(
        xs.reshape(n_sys, 128, 4, 2, 128).transpose(4, 0, 2, 3, 1).reshape(
            128, n_sys * 1024
        )
    ).astype(bf)
    return {
        "xt": xt,
        "wsb": wsb,
        "c8t": c8t,
        "linwt": linwt,
        "bias2": bias2,
    }


def _host_output(out):
    """Device output (bf16) -> final fp32."""
    return np.asarray(out).astype(np.float32)


_CACHE = {}


def kernel(
    x,
    expert_mixing_coefficients,
    routing_idxs,
    moe_weights,
    moe_bias,
    lin_weight,
    lin_bias,
    trace=False,
):
    from concourse.bass_utils import run_bass_kernel_spmd

    n_sys = B // NCORES
    if "nc" not in _CACHE:
        _CACHE["nc"] = build_program(n_sys)
    nc = _CACHE["nc"]
    in_maps = [
        _host_inputs(
            x, expert_mixing_coefficients, moe_weights, moe_bias, lin_weight, lin_bias,
            n_sys, c,
        )
        for c in range(NCORES)
    ]
    res = run_bass_kernel_spmd(nc, in_maps, list(range(NCORES)), trace=trace)
    outs = [res.results[c]["out"] for c in range(NCORES)]
    full = _host_output(np.concatenate(outs, axis=0))
    if trace:
        return full, res
    return full


# revision 38
# speedup vs baseline: 1.0149x; 1.0149x over previous
"""MOELinearDGLFractional Trainium2 kernel.

Data-parallel over systems: 8 cores x 64 systems (512 rows each).
All heavy tensors are bf16 (halves HBM traffic — memory-bound regime —
and runs PE at 1 cyc/row; fp32 accumulate in PSUM).

x is laid out transposed on the host (same byte count, free layout
choice): xt[i', s*1024 + (2j+h)*128 + q] = x[512 s + 4 q + j, 128 h + i'],
so each system's lhsT chunks stream straight from DRAM with contiguous
2KB per-partition lines and no on-device transpose pipeline.

Per system s (rows r = 512*s + 4*q + j, q=partition, j=0..3):
  - DMA xt tile [128, 1024]
  - fused matmul psum[q, (j%2)*256+o] += xt_k.T @ V[:, h*64+s, :]
    (k=2j+h; V = v3 [128(i'), hb, 256(o: 0-127 moe, 128-255 reg)])
  - DVE psum + bias -> out_sb (bf16), DMA out.
wmix (per-system mixed expert weights) produced on-PE in a prologue:
  one K=128-packed matmul per 8 (o,h)-chunks against block-diagonal coeff.
Regular Linear weights replicated into V's o=128..255 region by ScalarE.
"""

import sys

sys.path.insert(0, "/opt/trn_rl_repo")

import numpy as np

N_TOTAL = 262144
B = 512
E = 16
I_DIM = 256
O_MOE = 128
O_REG = 128
NCORES = 8
L = 512  # rows per system


def build_program(n_sys, reps=1):
    """reps>1 executes the full kernel (const DMAs + prologue + main loop)
    reps times inside one NEFF — used only for timing, where the slope over
    reps isolates true HW execution time from the 33-110ms axon dispatch
    latency (the neuronx_cc hook allows just one bass_exec custom call per
    XLA module, so repetition must live inside the BIR program)."""
    import concourse.bass as bass
    import concourse.mybir as mybir

    f32 = mybir.dt.float32
    bf16 = mybir.dt.bfloat16
    rows = n_sys * L
    hb = 2 * n_sys  # (h, b) combined dim of V
    nldw = 32  # number of K=128-packed ldw groups (256 chunks / 8)
    pw_n = 8 * n_sys  # prod psum free size per group (8 chunks x n_sys)

    BIO = 4  # systems per IO DMA (amortizes per-DMA fixed latency)
    nblk = n_sys // BIO
    NXB = 6  # xt_sb depth — deep enough that HBM stays saturated while
    #          the ~14us prologue runs (prefetch capacity > ramp window)
    NOB = 3  # o_sb depth (blocks)
    NPB = 4  # outp depth (2 banks each = all 8 PSUM banks)

    nc = bass.Bass()
    xt_d = nc.declare_dram_parameter("xt", [128, rows * 2], bf16, isOutput=False)
    wsb_d = nc.declare_dram_parameter("wsb", [128, 4096], bf16, isOutput=False)
    c8t_d = nc.declare_dram_parameter("c8t", [128, pw_n], bf16, isOutput=False)
    linwt_d = nc.declare_dram_parameter("linwt", [128, 256], bf16, isOutput=False)
    bias_d = nc.declare_dram_parameter("bias2", [128, 512], f32, isOutput=False)
    out = nc.declare_dram_parameter("out", [rows, 256], bf16, isOutput=True)

    # 4-system output blocks: per partition q, 4 contiguous-2KB segments
    # (one per system in the block) at 512-row stride
    ov4 = out.rearrange("(t sg q j) m -> t q sg (j m)", sg=BIO, q=128, j=4)

    from contextlib import ExitStack

    with ExitStack() as ctx:
        en = ctx.enter_context
        wsb = en(nc.sbuf_tensor("wsb_sb", [128, 4096], bf16))
        c8t = en(nc.sbuf_tensor("c8t_sb", [128, pw_n], bf16))
        linwt = en(nc.sbuf_tensor("linwt_sb", [128, 256], bf16))
        bias2 = en(nc.sbuf_tensor("bias2_sb", [128, 512], f32))
        # V: [i', hb, o(256: moe|reg)] — o contiguous so matmul moving
        # operands stream unit-stride
        v3 = en(nc.sbuf_tensor("v3_sb", [128, hb, 256], bf16))
        xt_sb = [
            en(nc.sbuf_tensor(f"xt_sb{i}", [128, BIO * 1024], bf16))
            for i in range(NXB)
        ]
        o_sb = [
            en(nc.sbuf_tensor(f"o_sb{i}", [128, BIO * 1024], bf16))
            for i in range(NOB)
        ]
        outp = [
            [en(nc.psum_tensor(f"outp{i}{k}", [128, 512], f32)) for k in range(2)]
            for i in range(NPB)
        ]

        sem_names = (
            ["cst", "mm", "dve", "pw", "pwe", "pve", "lws", "lwv"]
            + [f"xin{i}" for i in range(NXB)]
            + [f"dout{i}" for i in range(NOB)]
        )
        sems = {n: en(nc.semaphore(n)) for n in sem_names}
        cst_s, mm_s, dve_s, pw_s, pwe_s, pve_s = (
            sems[n] for n in ["cst", "mm", "dve", "pw", "pwe", "pve"]
        )
        lw_sems = {0: sems["lws"], 1: sems["lwv"]}
        lw_cnt = {0: 0, 1: 0}
        xin = [sems[f"xin{i}"] for i in range(NXB)]
        dout = [sems[f"dout{i}"] for i in range(NOB)]

        block = en(nc.Block())

        @block.sync
        def _(sync):
            for r in range(reps):
                if r > 0:
                    # rep r-1 fully drained (dve is the last const reader)
                    sync.wait_ge(dve_s, r * n_sys)
                sync.dma_start(out=wsb[:], in_=wsb_d[:]).then_inc(cst_s, 16)
                sync.dma_start(out=c8t[:], in_=c8t_d[:]).then_inc(cst_s, 16)
                sync.dma_start(out=linwt[:], in_=linwt_d[:]).then_inc(cst_s, 16)
                sync.dma_start(out=bias2[:], in_=bias_d[:]).then_inc(cst_s, 16)
                for tb in range(nblk):
                    t = r * nblk + tb
                    if t >= NXB:
                        # buffer free once the previous occupant's matmuls ran
                        sync.wait_ge(mm_s, BIO * (t - NXB + 1))
                    sync.dma_start(
                        out=xt_sb[t % NXB][:],
                        in_=xt_d[:, tb * BIO * 1024 : (tb + 1) * BIO * 1024],
                    ).then_inc(xin[t % NXB], 16)

        @block.tensor
        def _(tensor):
            for r in range(reps):
                # ---- prologue: produce V moe columns (mixed expert wts) ----
                tensor.wait_ge(cst_s, 64 * (r + 1))
                for g in range(nldw):
                    gg = r * nldw + g
                    if gg >= NPB:
                        # bank free once group gg-NPB was evacuated (even
                        # groups on ScalarE -> pwe, odd on VectorE -> pve)
                        k = gg - NPB
                        tensor.wait_ge(pwe_s if k % 2 == 0 else pve_s, k // 2 + 1)
                    if r > 0 and g < NPB:
                        # outp banks still read by rep r-1's bias adds
                        tensor.wait_ge(dve_s, r * n_sys)
                    pp = outp[gg % NPB][0]
                    for v in range(8):
                        inst = nc.tensor.matmul(
                            pp[:, v * n_sys : (v + 1) * n_sys],
                            wsb[:, g * 128 : (g + 1) * 128],
                            c8t[:, v * n_sys : (v + 1) * n_sys],
                            start=True,
                            stop=True,
                        )
                    inst.then_inc(pw_s, 1)

                # ---- main loop: 8 fused matmuls per system ----
                for s in range(n_sys):
                    g = r * n_sys + s
                    t = g // BIO
                    if g % BIO == 0:
                        tensor.wait_ge(xin[t % NXB], 16 * (t // NXB + 1))
                    if s == 0:
                        tensor.wait_ge(pwe_s, (r + 1) * nldw // 2)
                        tensor.wait_ge(pve_s, (r + 1) * nldw // 2)
                    if g >= NPB:
                        tensor.wait_ge(dve_s, g - (NPB - 1))
                    xb = xt_sb[t % NXB]
                    xo = (g % BIO) * 1024
                    for j in range(4):
                        pp = outp[g % NPB][j // 2]
                        for h in range(2):
                            inst = nc.tensor.matmul(
                                pp[:, (j % 2) * 256 : (j % 2) * 256 + 256],
                                xb[:, xo + (2 * j + h) * 128 : xo + (2 * j + h + 1) * 128],
                                v3[:, bass.ds(h * n_sys + s, 1), :],
                                start=(h == 0),
                                stop=(h == 1),
                            )
                    inst.then_inc(mm_s, 1)

        def linwt_fill(eng, cp, h):
            # seed + log-doubling of linwt into v3[:, h*n+b, 128:256] over b;
            # a same-engine semaphore chain orders each step's RAW hazard
            sem = lw_sems[h]
            cp(
                out=v3[:, bass.ds(h * n_sys, 1), 128:256],
                in_=linwt[:, h * 128 : (h + 1) * 128],
            ).then_inc(sem, 1)
            lw_cnt[h] += 1
            k = 1
            while k < n_sys:
                kk = min(k, n_sys - k)
                eng.wait_ge(sem, lw_cnt[h])
                cp(
                    out=v3[:, h * n_sys + k : h * n_sys + k + kk, 128:256],
                    in_=v3[:, h * n_sys : h * n_sys + kk, 128:256],
                ).then_inc(sem, 1)
                lw_cnt[h] += 1
                k *= 2

        def evac_group(cp, r, g):
            # group g holds chunks c=8g+v = 2o+h (o = 4g+o2): psum free
            # (o2, h, b) with (h, b) contiguous -> one dim-permuted copy
            pp = outp[(r * nldw + g) % NPB][0]
            return cp(
                out=v3[:, :, 4 * g : 4 * g + 4],
                in_=pp[:, 0 : 8 * n_sys].rearrange("p (o2 hb) -> p hb o2", o2=4),
            )

        @block.scalar
        def _(scalar):
            # prologue work is split with VectorE (idle until matmuls flow):
            # ScalarE takes the h=0 linwt fill + even evac groups
            for r in range(reps):
                scalar.wait_ge(cst_s, 64 * (r + 1))
                if r > 0:
                    # v3 still read by rep r-1's matmuls
                    scalar.wait_ge(mm_s, r * n_sys)
                linwt_fill(scalar, nc.scalar.copy, 0)
                for g in range(0, nldw, 2):
                    scalar.wait_ge(pw_s, r * nldw + g + 1)
                    evac_group(nc.scalar.copy, r, g).then_inc(pwe_s, 1)

        @block.vector
        def _(vector):
            for r in range(reps):
                # prologue: h=1 linwt fill + odd evac groups (v3 free: our
                # own rep r-1 adds already waited out all its matmuls)
                vector.wait_ge(cst_s, 64 * (r + 1))
                linwt_fill(vector, nc.vector.tensor_copy, 1)
                for g in range(1, nldw, 2):
                    vector.wait_ge(pw_s, r * nldw + g + 1)
                    evac_group(nc.vector.tensor_copy, r, g).then_inc(pve_s, 1)
                # main: bias adds
                for s in range(n_sys):
                    g = r * n_sys + s
                    vector.wait_ge(mm_s, g + 1)
                    t = g // BIO
                    if g % BIO == 0 and t >= NOB:
                        vector.wait_ge(dout[t % NOB], 16 * (t // NOB))
                    pb = g % NPB
                    ob = o_sb[t % NOB]
                    oo = (g % BIO) * 1024
                    nc.vector.tensor_add(
                        ob[:, oo : oo + 512], outp[pb][0][:], bias2[:]
                    )
                    nc.vector.tensor_add(
                        ob[:, oo + 512 : oo + 1024], outp[pb][1][:], bias2[:]
                    ).then_inc(dve_s, 1)

        @block.gpsimd
        def _(gpsimd):
            # main: out DMA per block of BIO systems
            tblk = reps * nblk
            for t in range(tblk):
                gpsimd.wait_ge(dve_s, BIO * (t + 1))
                gpsimd.dma_start(
                    out=ov4[t % nblk], in_=o_sb[t % NOB][:]
                ).then_inc(dout[t % NOB], 16)
            for i in range(NOB):
                gpsimd.wait_ge(dout[i], 16 * ((tblk - i + NOB - 1) // NOB))

    return nc


def _bf16():
    import ml_dtypes

    return ml_dtypes.bfloat16


def _host_inputs(x, coeff, moe_weights, moe_bias, lin_weight, lin_bias, n_sys, core):
    """Build per-core in_map."""
    bf = _bf16()
    # wsb: [16v+e, 128g+i'] = W[e, o(c), 128h(c)+i'], c=8g+v, (o,h)=divmod(c,2)
    Wr = np.asarray(moe_weights).reshape(E, 128, 2, 128)  # e,o,h,i'
    ch = Wr.transpose(1, 2, 0, 3).reshape(256, E, 128)  # c,e,i'
    wsb = np.ascontiguousarray(
        ch.reshape(32, 8, E, 128).transpose(1, 2, 0, 3).reshape(128, 4096)
    ).astype(bf)
    b0 = core * n_sys
    cT = np.asarray(coeff)[b0 : b0 + n_sys].T.astype(np.float32)  # [E, n_sys]
    c8t = np.zeros((128, 8 * n_sys), np.float32)
    for v in range(8):
        c8t[16 * v : 16 * v + E, v * n_sys : (v + 1) * n_sys] = cT
    c8t = c8t.astype(bf)
    lw = np.asarray(lin_weight)  # [128, 256]
    linwt = np.ascontiguousarray(
        lw.reshape(128, 2, 128).transpose(2, 1, 0).reshape(128, 256)
    ).astype(bf)
    bias_cat = np.concatenate([np.asarray(moe_bias), np.asarray(lin_bias)]).astype(
        np.float32
    )
    bias2 = np.tile(bias_cat, (128, 2))
    # xt[i', (s, j, h, q)] = x[512 s + 4 q + j, 128 h + i']
    xs = np.asarray(x)[core * n_sys * L : (core + 1) * n_sys * L]
    xt = np.ascontiguousarray(
        xs.reshape(n_sys, 128, 4, 2, 128).transpose(4, 0, 2, 3, 1).reshape(
            128, n_sys * 1024
        )
    ).astype(bf)
    return {
        "xt": xt,
        "wsb": wsb,
        "c8t": c8t,
        "linwt": linwt,
        "bias2": bias2,
    }


def _host_output(out):
    """Device output (bf16) -> final fp32."""
    return np.asarray(out).astype(np.float32)


_CACHE = {}


def kernel(
    x,
    expert_mixing_coefficients,
    routing_idxs,
    moe_weights,
    moe_bias,
    lin_weight,
    lin_bias,
    trace=False,
):
    from concourse.bass_utils import run_bass_kernel_spmd

    n_sys = B // NCORES
    if "nc" not in _CACHE:
        _CACHE["nc"] = build_program(n_sys)
    nc = _CACHE["nc"]
    in_maps = [
        _host_inputs(
            x, expert_mixing_coefficients, moe_weights, moe_bias, lin_weight, lin_bias,
            n_sys, c,
        )
        for c in range(NCORES)
    ]
    res = run_bass_kernel_spmd(nc, in_maps, list(range(NCORES)), trace=trace)
    outs = [res.results[c]["out"] for c in range(NCORES)]
    full = _host_output(np.concatenate(outs, axis=0))
    if trace:
        return full, res
    return full


# revision 39
# speedup vs baseline: 1.0493x; 1.0339x over previous
"""MOELinearDGLFractional Trainium2 kernel.

Data-parallel over systems: 8 cores x 64 systems (512 rows each).
All heavy tensors are bf16 (halves HBM traffic — memory-bound regime —
and runs PE at 1 cyc/row; fp32 accumulate in PSUM).

x is laid out transposed on the host (same byte count, free layout
choice): xt[i', s*1024 + (2j+h)*128 + q] = x[512 s + 4 q + j, 128 h + i'],
so each system's lhsT chunks stream straight from DRAM with contiguous
2KB per-partition lines and no on-device transpose pipeline.

Per system s (rows r = 512*s + 4*q + j, q=partition, j=0..3):
  - DMA xt tile [128, 1024]
  - fused matmul psum[q, (j%2)*256+o] += xt_k.T @ V[:, h*64+s, :]
    (k=2j+h; V = v3 [128(i'), hb, 256(o: 0-127 moe, 128-255 reg)])
  - DVE psum + bias -> out_sb (bf16), DMA out.
wmix (per-system mixed expert weights) produced on-PE in a prologue:
  one K=128-packed matmul per 8 (o,h)-chunks against block-diagonal coeff.
Regular Linear weights replicated into V's o=128..255 region by ScalarE.
"""

import sys

sys.path.insert(0, "/opt/trn_rl_repo")

import numpy as np

N_TOTAL = 262144
B = 512
E = 16
I_DIM = 256
O_MOE = 128
O_REG = 128
NCORES = 8
L = 512  # rows per system


def build_program(n_sys, reps=1):
    """reps>1 executes the full kernel (const DMAs + prologue + main loop)
    reps times inside one NEFF — used only for timing, where the slope over
    reps isolates true HW execution time from the 33-110ms axon dispatch
    latency (the neuronx_cc hook allows just one bass_exec custom call per
    XLA module, so repetition must live inside the BIR program)."""
    import concourse.bass as bass
    import concourse.mybir as mybir

    f32 = mybir.dt.float32
    bf16 = mybir.dt.bfloat16
    rows = n_sys * L
    hb = 2 * n_sys  # (h, b) combined dim of V
    nldw = 32  # number of K=128-packed ldw groups (256 chunks / 8)
    pw_n = 8 * n_sys  # prod psum free size per group (8 chunks x n_sys)

    BIO = 4  # systems per IO DMA (amortizes per-DMA fixed latency)
    nblk = n_sys // BIO
    NXB = 6  # xt_sb depth — deep enough that HBM stays saturated while
    #          the ~14us prologue runs (prefetch capacity > ramp window)
    NOB = 6  # o_sb depth — mirror the input side: deep enough that DVE
    #          never stalls on out-DMA turnaround
    NPB = 4  # outp depth (2 banks each = all 8 PSUM banks)

    nc = bass.Bass()
    xt_d = nc.declare_dram_parameter("xt", [128, rows * 2], bf16, isOutput=False)
    wsb_d = nc.declare_dram_parameter("wsb", [128, 4096], bf16, isOutput=False)
    c8t_d = nc.declare_dram_parameter("c8t", [128, pw_n], bf16, isOutput=False)
    linwt_d = nc.declare_dram_parameter("linwt", [128, 256], bf16, isOutput=False)
    bias_d = nc.declare_dram_parameter("bias2", [128, 512], f32, isOutput=False)
    out = nc.declare_dram_parameter("out", [rows, 256], bf16, isOutput=True)

    # 4-system output blocks: per partition q, 4 contiguous-2KB segments
    # (one per system in the block) at 512-row stride
    ov4 = out.rearrange("(t sg q j) m -> t q sg (j m)", sg=BIO, q=128, j=4)

    from contextlib import ExitStack

    with ExitStack() as ctx:
        en = ctx.enter_context
        wsb = en(nc.sbuf_tensor("wsb_sb", [128, 4096], bf16))
        c8t = en(nc.sbuf_tensor("c8t_sb", [128, pw_n], bf16))
        linwt = en(nc.sbuf_tensor("linwt_sb", [128, 256], bf16))
        bias2 = en(nc.sbuf_tensor("bias2_sb", [128, 512], f32))
        # V: [i', hb, o(256: moe|reg)] — o contiguous so matmul moving
        # operands stream unit-stride
        v3 = en(nc.sbuf_tensor("v3_sb", [128, hb, 256], bf16))
        xt_sb = [
            en(nc.sbuf_tensor(f"xt_sb{i}", [128, BIO * 1024], bf16))
            for i in range(NXB)
        ]
        o_sb = [
            en(nc.sbuf_tensor(f"o_sb{i}", [128, BIO * 1024], bf16))
            for i in range(NOB)
        ]
        outp = [
            [en(nc.psum_tensor(f"outp{i}{k}", [128, 512], f32)) for k in range(2)]
            for i in range(NPB)
        ]

        sem_names = (
            ["cst", "mm", "dve", "pw", "pwe", "pve", "lws", "lwv"]
            + [f"xin{i}" for i in range(NXB)]
            + [f"dout{i}" for i in range(NOB)]
        )
        sems = {n: en(nc.semaphore(n)) for n in sem_names}
        cst_s, mm_s, dve_s, pw_s, pwe_s, pve_s = (
            sems[n] for n in ["cst", "mm", "dve", "pw", "pwe", "pve"]
        )
        lw_sems = {0: sems["lws"], 1: sems["lwv"]}
        lw_cnt = {0: 0, 1: 0}
        xin = [sems[f"xin{i}"] for i in range(NXB)]
        dout = [sems[f"dout{i}"] for i in range(NOB)]

        block = en(nc.Block())

        @block.sync
        def _(sync):
            for r in range(reps):
                if r > 0:
                    # rep r-1 fully drained (dve is the last const reader)
                    sync.wait_ge(dve_s, r * n_sys)
                sync.dma_start(out=wsb[:], in_=wsb_d[:]).then_inc(cst_s, 16)
                sync.dma_start(out=c8t[:], in_=c8t_d[:]).then_inc(cst_s, 16)
                sync.dma_start(out=linwt[:], in_=linwt_d[:]).then_inc(cst_s, 16)
                sync.dma_start(out=bias2[:], in_=bias_d[:]).then_inc(cst_s, 16)
                for tb in range(nblk):
                    t = r * nblk + tb
                    if t >= NXB:
                        # buffer free once the previous occupant's matmuls ran
                        sync.wait_ge(mm_s, BIO * (t - NXB + 1))
                    sync.dma_start(
                        out=xt_sb[t % NXB][:],
                        in_=xt_d[:, tb * BIO * 1024 : (tb + 1) * BIO * 1024],
                    ).then_inc(xin[t % NXB], 16)

        @block.tensor
        def _(tensor):
            for r in range(reps):
                # ---- prologue: produce V moe columns (mixed expert wts) ----
                tensor.wait_ge(cst_s, 64 * (r + 1))
                for g in range(nldw):
                    gg = r * nldw + g
                    if gg >= NPB:
                        # bank free once group gg-NPB was evacuated (even
                        # groups on ScalarE -> pwe, odd on VectorE -> pve)
                        k = gg - NPB
                        tensor.wait_ge(pwe_s if k % 2 == 0 else pve_s, k // 2 + 1)
                    if r > 0 and g < NPB:
                        # outp banks still read by rep r-1's bias adds
                        tensor.wait_ge(dve_s, r * n_sys)
                    pp = outp[gg % NPB][0]
                    for v in range(8):
                        inst = nc.tensor.matmul(
                            pp[:, v * n_sys : (v + 1) * n_sys],
                            wsb[:, g * 128 : (g + 1) * 128],
                            c8t[:, v * n_sys : (v + 1) * n_sys],
                            start=True,
                            stop=True,
                        )
                    inst.then_inc(pw_s, 1)

                # ---- main loop: 8 fused matmuls per system ----
                for s in range(n_sys):
                    g = r * n_sys + s
                    t = g // BIO
                    if g % BIO == 0:
                        tensor.wait_ge(xin[t % NXB], 16 * (t // NXB + 1))
                    if s == 0:
                        tensor.wait_ge(pwe_s, (r + 1) * nldw // 2)
                        tensor.wait_ge(pve_s, (r + 1) * nldw // 2)
                    if g >= NPB:
                        tensor.wait_ge(dve_s, g - (NPB - 1))
                    xb = xt_sb[t % NXB]
                    xo = (g % BIO) * 1024
                    for j in range(4):
                        pp = outp[g % NPB][j // 2]
                        for h in range(2):
                            inst = nc.tensor.matmul(
                                pp[:, (j % 2) * 256 : (j % 2) * 256 + 256],
                                xb[:, xo + (2 * j + h) * 128 : xo + (2 * j + h + 1) * 128],
                                v3[:, bass.ds(h * n_sys + s, 1), :],
                                start=(h == 0),
                                stop=(h == 1),
                            )
                    inst.then_inc(mm_s, 1)

        def linwt_fill(eng, cp, h):
            # seed + log-doubling of linwt into v3[:, h*n+b, 128:256] over b;
            # a same-engine semaphore chain orders each step's RAW hazard
            sem = lw_sems[h]
            cp(
                out=v3[:, bass.ds(h * n_sys, 1), 128:256],
                in_=linwt[:, h * 128 : (h + 1) * 128],
            ).then_inc(sem, 1)
            lw_cnt[h] += 1
            k = 1
            while k < n_sys:
                kk = min(k, n_sys - k)
                eng.wait_ge(sem, lw_cnt[h])
                cp(
                    out=v3[:, h * n_sys + k : h * n_sys + k + kk, 128:256],
                    in_=v3[:, h * n_sys : h * n_sys + kk, 128:256],
                ).then_inc(sem, 1)
                lw_cnt[h] += 1
                k *= 2

        def evac_group(cp, r, g):
            # group g holds chunks c=8g+v = 2o+h (o = 4g+o2): psum free
            # (o2, h, b) with (h, b) contiguous -> one dim-permuted copy
            pp = outp[(r * nldw + g) % NPB][0]
            return cp(
                out=v3[:, :, 4 * g : 4 * g + 4],
                in_=pp[:, 0 : 8 * n_sys].rearrange("p (o2 hb) -> p hb o2", o2=4),
            )

        @block.scalar
        def _(scalar):
            # prologue work is split with VectorE (idle until matmuls flow):
            # ScalarE takes the h=0 linwt fill + even evac groups
            for r in range(reps):
                scalar.wait_ge(cst_s, 64 * (r + 1))
                if r > 0:
                    # v3 still read by rep r-1's matmuls
                    scalar.wait_ge(mm_s, r * n_sys)
                linwt_fill(scalar, nc.scalar.copy, 0)
                for g in range(0, nldw, 2):
                    scalar.wait_ge(pw_s, r * nldw + g + 1)
                    evac_group(nc.scalar.copy, r, g).then_inc(pwe_s, 1)

        @block.vector
        def _(vector):
            for r in range(reps):
                # prologue: h=1 linwt fill + odd evac groups (v3 free: our
                # own rep r-1 adds already waited out all its matmuls)
                vector.wait_ge(cst_s, 64 * (r + 1))
                linwt_fill(vector, nc.vector.tensor_copy, 1)
                for g in range(1, nldw, 2):
                    vector.wait_ge(pw_s, r * nldw + g + 1)
                    evac_group(nc.vector.tensor_copy, r, g).then_inc(pve_s, 1)
                # main: bias adds
                for s in range(n_sys):
                    g = r * n_sys + s
                    vector.wait_ge(mm_s, g + 1)
                    t = g // BIO
                    if g % BIO == 0 and t >= NOB:
                        vector.wait_ge(dout[t % NOB], 16 * (t // NOB))
                    pb = g % NPB
                    ob = o_sb[t % NOB]
                    oo = (g % BIO) * 1024
                    nc.vector.tensor_add(
                        ob[:, oo : oo + 512], outp[pb][0][:], bias2[:]
                    )
                    nc.vector.tensor_add(
                        ob[:, oo + 512 : oo + 1024], outp[pb][1][:], bias2[:]
                    ).then_inc(dve_s, 1)

        @block.gpsimd
        def _(gpsimd):
            # main: out DMA per block of BIO systems
            tblk = reps * nblk
            for t in range(tblk):
                gpsimd.wait_ge(dve_s, BIO * (t + 1))
                gpsimd.dma_start(
                    out=ov4[t % nblk], in_=o_sb[t % NOB][:]
                ).then_inc(dout[t % NOB], 16)
            for i in range(NOB):
                gpsimd.wait_ge(dout[i], 16 * ((tblk - i + NOB - 1) // NOB))

    return nc


def _bf16():
    import ml_dtypes

    return ml_dtypes.bfloat16


def _host_inputs(x, coeff, moe_weights, moe_bias, lin_weight, lin_bias, n_sys, core):
    """Build per-core in_map."""
    bf = _bf16()
    # wsb: [16v+e, 128g+i'] = W[e, o(c), 128h(c)+i'], c=8g+v, (o,h)=divmod(c,2)
    Wr = np.asarray(moe_weights).reshape(E, 128, 2, 128)  # e,o,h,i'
    ch = Wr.transpose(1, 2, 0, 3).reshape(256, E, 128)  # c,e,i'
    wsb = np.ascontiguousarray(
        ch.reshape(32, 8, E, 128).transpose(1, 2, 0, 3).reshape(128, 4096)
    ).astype(bf)
    b0 = core * n_sys
    cT = np.asarray(coeff)[b0 : b0 + n_sys].T.astype(np.float32)  # [E, n_sys]
    c8t = np.zeros((128, 8 * n_sys), np.float32)
    for v in range(8):
        c8t[16 * v : 16 * v + E, v * n_sys : (v + 1) * n_sys] = cT
    c8t = c8t.astype(bf)
    lw = np.asarray(lin_weight)  # [128, 256]
    linwt = np.ascontiguousarray(
        lw.reshape(128, 2, 128).transpose(2, 1, 0).reshape(128, 256)
    ).astype(bf)
    bias_cat = np.concatenate([np.asarray(moe_bias), np.asarray(lin_bias)]).astype(
        np.float32
    )
    bias2 = np.tile(bias_cat, (128, 2))
    # xt[i', (s, j, h, q)] = x[512 s + 4 q + j, 128 h + i']
    xs = np.asarray(x)[core * n_sys * L : (core + 1) * n_sys * L]
    xt = np.ascontiguousarray(
        xs.reshape(n_sys, 128, 4, 2, 128).transpose(4, 0, 2, 3, 1).reshape(
            128, n_sys * 1024
        )
    ).astype(bf)
    return {
        "xt": xt,
        "wsb": wsb,
        "c8t": c8t,
        "linwt": linwt,
        "bias2": bias2,
    }


def _host_output(out):
    """Device output (bf16) -> final fp32."""
    return np.asarray(out).astype(np.float32)


_CACHE = {}


def kernel(
    x,
    expert_mixing_coefficients,
    routing_idxs,
    moe_weights,
    moe_bias,
    lin_weight,
    lin_bias,
    trace=False,
):
    from concourse.bass_utils import run_bass_kernel_spmd

    n_sys = B // NCORES
    if "nc" not in _CACHE:
        _CACHE["nc"] = build_program(n_sys)
    nc = _CACHE["nc"]
    in_maps = [
        _host_inputs(
            x, expert_mixing_coefficients, moe_weights, moe_bias, lin_weight, lin_bias,
            n_sys, c,
        )
        for c in range(NCORES)
    ]
    res = run_bass_kernel_spmd(nc, in_maps, list(range(NCORES)), trace=trace)
    outs = [res.results[c]["out"] for c in range(NCORES)]
    full = _host_output(np.concatenate(outs, axis=0))
    if trace:
        return full, res
    return full
